# revision 50
# baseline (speedup 1.0000x reference)
"""Tensor-parallel multi-head attention for 8 Trainium2 NeuronCores.

Sharding (TP8 over heads): core c owns heads {2c, 2c+1} (128 q/k/v features)
and computes them for BOTH batch elements; out_proj is column-sharded with
8-core mesh AllGathers of the context shards, one per (batch, seq-half),
each emitted as soon as that half's context is normalized so the serialized
CC stream overlaps remaining attention work.

Per-core dataflow (activations kept transposed, [feature, token]):
  qT/kT/vT = W.T-chunks @ xT          (PE, bf16, fp32 PSUM accum)
  v        = PE-transpose(vT)          (with an appended ones-column)
  sT[k,q]  = kT-block.T @ qT           (causal: upper-right blocks skipped)
  aT       = exp(sT/8 + mask_bias)     (ACT from PSUM; safe without
                                        max-subtraction: scores ~ N(0,1))
  ctxT;sum = [v|1].T @ aT              (ones row gives the softmax denom)
  ctxT    *= 1/sum                     (per-q-block, as soon as its k-loop
                                        completes; reciprocal_approx_fast)

Scheduling: the attention stream is ACT(exp)-bound in steady state, so all
other PE work is threaded INTO it to keep the Tensor engine dense (the HAM
clock-gate stays at 2.4 GHz): batch-1's projections and v-transposes are
interleaved into batch-0's attention; batch-0's out-proj halves into
batch-1's attention, placed late enough that their gathers have completed
(engine queues are FIFO, so a premature out-proj matmul would block all
attention work queued behind it).  The ctx matmuls of k-chunk step N are
emitted ~1 step behind its score matmuls, giving the Scalar engine time to
produce exp(scores) before the Tensor engine's queue reaches the ctx
matmul that consumes them.  The two heads are zipped 1:1 so consecutive
score matmuls alternate h0/h64 row-groups (concurrent PE subarrays).
Host side only reshapes/concatenates shards (dtype prep of inputs aside).
The gathered feature order is [core, local-head, dh]; the host permutes
wo's input dimension to match.
"""

import sys
from collections import deque

for _p in ("/opt/trn_rl_repo",):
    if _p not in sys.path:
        sys.path.append(_p)

import numpy as np
import ml_dtypes

import concourse.bass as bass  # noqa: F401
import concourse.mybir as mybir
import concourse.tile as tile
from concourse import bacc, bass_utils
from concourse.masks import make_identity, make_upper_triangular

BF16 = mybir.dt.bfloat16
FP8 = mybir.dt.float8e4
F32 = mybir.dt.float32
F32R = mybir.dt.float32r
Exp = mybir.ActivationFunctionType.Exp
DR = mybir.MatmulPerfMode.DoubleRow

# q/k/v weights are shipped x16 in fp8 (lifts them off the e4m3 denormal
# floor); scores scale by 256, ctx by 16 -- folded into the exp scale and
# the host-side woT scaling.
WSCALE = 16.0

B, S, D = 2, 2048, 1024
T = B * S            # 4096 tokens across batches
H, DH = 16, 64
NCORES = 8
HPC = H // NCORES    # heads per core = 2
F = HPC * DH         # features per core = 128
KC = S // 128        # 16 k-chunks per batch
QB = S // 512        # 4 q-blocks of 512 per batch

_CACHED = {}


def _build(with_bias: bool):
    nc = bacc.Bacc(
        "TRN2",
        target_bir_lowering=False,
        debug=False,
        enable_asserts=True,
        num_devices=NCORES,
    )
    xT_d = nc.dram_tensor("xT", [D, T], BF16, kind="ExternalInput").ap()
    wqT_d = nc.dram_tensor("wqT", [D, F], BF16, kind="ExternalInput").ap()
    wkT_d = nc.dram_tensor("wkT", [D, F], BF16, kind="ExternalInput").ap()
    wvT_d = nc.dram_tensor("wvT", [D, F], BF16, kind="ExternalInput").ap()
    woT_d = nc.dram_tensor("woT", [D, F], BF16, kind="ExternalInput").ap()
    b_d = {}
    if with_bias:
        for nm in ("bq", "bk", "bv", "bo"):
            b_d[nm] = nc.dram_tensor(nm, [1, F], BF16, kind="ExternalInput").ap()
    maskb_d = nc.dram_tensor("maskb", [128, B * KC], F32, kind="ExternalInput").ap()
    outT_d = nc.dram_tensor("outT", [F, T], BF16, kind="ExternalOutput").ap()

    with tile.TileContext(nc) as tc:
        with (
            tc.tile_pool(name="singles", bufs=1) as sg,
            tc.tile_pool(name="att", bufs=6) as att_pool,
            tc.tile_pool(name="out", bufs=2) as out_pool,
            tc.tile_pool(name="cf", bufs=2) as cf_pool,
            tc.tile_pool(name="psA", bufs=2, space="PSUM") as psA,
            tc.tile_pool(name="psB", bufs=4, space="PSUM") as psB,
            tc.tile_pool(name="dram", bufs=1, space="DRAM") as dram,
        ):
            # ---- constants -------------------------------------------------
            ident = sg.tile([128, 128], BF16, name="ident")
            make_identity(nc, ident)
            trimask = sg.tile([128, 128], BF16, name="trimask")
            make_upper_triangular(nc, trimask, val=1.0, diag=True)
            trimask8 = sg.tile([128, 128], FP8, name="trimask8")
            nc.vector.tensor_copy(trimask8, trimask)
            ones64f = sg.tile([1, 64], F32, name="ones64f")
            nc.vector.memset(ones64f, 1.0)
            ones64r = sg.tile([1, 64], F32R, name="ones64r")
            nc.vector.tensor_copy(ones64r, ones64f)
            if with_bias:
                ones512 = sg.tile([1, 512], BF16, name="ones512")
                nc.vector.memset(ones512, 1.0)

            # ---- load inputs (split for early start) -----------------------
            maskb_sb = sg.tile([128, B * KC], F32, name="maskb_sb")
            nc.sync.dma_start(maskb_sb, maskb_d)
            w_sb = {}
            for nm, dd in (("v", wvT_d), ("k", wkT_d), ("q", wqT_d), ("o", woT_d)):
                w_sb[nm] = sg.tile([128, 8, F], BF16, name=f"w{nm}T_sb")
                nc.sync.dma_start(w_sb[nm], dd.rearrange("(o p) f -> p o f", p=128))
            # tiny warm-up collective: absorbs the one-time mesh-entry
            # barrier (~36us) while the projections run, so the first real
            # gather isn't delayed by it
            warm_in = dram.tile([128, 8], BF16, name="warm_in")
            warm_out = dram.tile(
                [NCORES * 128, 8], BF16, addr_space="Shared", name="warm_out"
            )
            nc.gpsimd.collective_compute(
                "AllGather",
                mybir.AluOpType.bypass,
                replica_groups=[list(range(NCORES))],
                ins=[warm_in.opt()],
                outs=[warm_out.opt()],
            )
            b_sb = {}
            if with_bias:
                for nm in ("bq", "bk", "bv", "bo"):
                    b_sb[nm] = sg.tile([1, F], BF16, name=f"{nm}_sb")
                    nc.sync.dma_start(b_sb[nm], b_d[nm])

            # persistent tiles first; xT halves last (freed first: LIFO).
            qT_sb, qT_free = tc.tile([128, T], BF16, name="qT_sb")
            kT_sb, kT_free = tc.tile([128, T], BF16, name="kT_sb")
            ctxT_sb, ctxT_free = tc.tile([64, HPC, T], BF16, name="ctxT_sb")
            vT_sb, vT_free = tc.tile([128, T], BF16, name="vT_sb")
            xT_sb = {}
            xT_frees = []
            for b in (1, 0):
                xT_sb[b], f_ = tc.tile([128, 8, S], BF16, name=f"xT_sb{b}")
                xT_frees.append(f_)
            xT_r = xT_d.rearrange("(o p) f -> p o f", p=128)
            # ki-major per half so the ki-ordered projection consumes the
            # stream as it lands; batch-0 halves first (projections for
            # batch 1 run later, inside batch-0's attention)
            for half in range(4):
                for ki in range(8):
                    cs = (half % 2) * 1024
                    nc.sync.dma_start(
                        xT_sb[half // 2][:, ki, cs:cs + 1024],
                        xT_r[:, ki, half * 1024:half * 1024 + 1024],
                    )

            v_ones = sg.tile([128, B * KC, HPC, DH + 1], BF16, name="v_ones")
            nc.vector.memset(v_ones, 1.0)

            # ---- projection / transpose chunks (emitted piecemeal) ---------
            def project(which, half):
                """One [128, 1024] slice of one projection: 8 ki-chunks."""
                w = w_sb[which]
                dst = {"v": vT_sb, "k": kT_sb, "q": qT_sb}[which]
                ps = psA.tile([128, 1024], F32, tag="work", name=f"p_{which}_{half}")
                for ki in range(8):
                    for nb in range(2):
                        cs = (half % 2) * 1024 + nb * 512
                        nc.tensor.matmul(
                            ps[:, nb * 512:nb * 512 + 512],
                            lhsT=w[:, ki, :],
                            rhs=xT_sb[half // 2][:, ki, cs:cs + 512],
                            start=(ki == 0),
                            stop=(ki == 7 and not with_bias),
                        )
                if with_bias:
                    bias = b_sb["b" + which]
                    for nb in range(2):
                        nc.tensor.matmul(
                            ps[:, nb * 512:nb * 512 + 512],
                            lhsT=bias[0:1, :],
                            rhs=ones512[0:1, :],
                            start=False,
                            stop=True,
                        )
                nc.vector.tensor_copy(dst[:, half * 1024:half * 1024 + 1024], ps)

            def vt_chunk(tb0):
                """Transpose 8 vT 128-blocks into v_ones rows."""
                pt = psA.tile([128, 1024], BF16, tag="work", name=f"vt_{tb0}")
                for i in range(8):
                    tb = tb0 + i
                    nc.tensor.transpose(
                        pt[:, i * 128:i * 128 + 128],
                        vT_sb[:, tb * 128:tb * 128 + 128],
                        ident,
                    )
                for i in range(8):
                    tb = tb0 + i
                    for h in range(HPC):
                        nc.vector.tensor_copy(
                            v_ones[:, tb, h, 0:DH],
                            pt[:, i * 128 + h * 64:i * 128 + h * 64 + 64],
                        )

            # ---- attention -------------------------------------------------
            sums_r = sg.tile([1, 2 * S], F32R, name="sums_r")
            rec_sb = sg.tile([64, 1024], F32, name="rec_sb")

            def normalize_qb(h, b, qb, ctx_tile):
                t0 = b * S
                so = h * S + qb * 512
                ro = h * 512
                nc.vector.tensor_copy(
                    sums_r[0:1, so:so + 512], ctx_tile[DH:DH + 1, :]
                )
                bc = psA.tile(
                    [128, 512], F32, tag="work", name=f"bc_{h}_{b}_{qb}"
                )
                nc.tensor.matmul(
                    bc[0:64, :],
                    lhsT=ones64r[0:1, :],
                    rhs=sums_r[0:1, so:so + 512],
                    start=True,
                    stop=True,
                )
                nc.vector.reciprocal_approx_fast(
                    rec_sb[:, ro:ro + 512], bc[0:64, :]
                )
                nc.vector.tensor_mul(
                    ctxT_sb[:, h, t0 + qb * 512:t0 + qb * 512 + 512],
                    ctx_tile[0:DH, :],
                    rec_sb[:, ro:ro + 512],
                )

            def scores_part(h, b, kc, qlo, qhi):
                """Emit score matmuls + exp for one k-chunk; returns the
                attention-weights tile for the ctx part."""
                po = 64 * h
                t0 = b * S
                q0 = kc * 128
                lo = max(q0, qlo)
                w = qhi - lo
                kT_blk = kT_sb[po:po + 64, t0 + q0:t0 + q0 + 128]
                st = psA.tile(
                    [128, 1024], F32, tag="work", name=f"st_{h}_{b}_{kc}_{qlo}"
                )
                c = lo
                while c < qhi:
                    c2 = min(qhi, (c // 512 + 1) * 512)
                    nc.tensor.matmul(
                        st[:, c - qlo:c2 - qlo],
                        lhsT=kT_blk,
                        rhs=qT_sb[po:po + 64, t0 + c:t0 + c2],
                        start=True,
                        stop=True,
                    )
                    c = c2
                at = att_pool.tile([128, 1024], BF16, tag="att")
                nc.scalar.activation(
                    at[:, 0:w],
                    st[:, lo - qlo:qhi - qlo],
                    Exp,
                    bias=maskb_sb[:, b * KC + kc:b * KC + kc + 1],
                    scale=0.125,
                )
                if lo == q0:  # diagonal 128-block: causal interior
                    nc.vector.tensor_mul(at[:, 0:128], at[:, 0:128], trimask)
                return at

            def ctx_part(h, b, kc, qlo, qhi, ctx_ps, at):
                t0 = b * S
                q0 = kc * 128
                lo = max(q0, qlo)
                c = lo
                while c < qhi:
                    qb = c // 512
                    c2 = min(qhi, (qb + 1) * 512)
                    nc.tensor.matmul(
                        ctx_ps[qb][0:DH + 1, c - qb * 512:c2 - qb * 512],
                        lhsT=v_ones[:, b * KC + kc, h, :],
                        rhs=at[:, c - lo:c2 - lo],
                        start=(kc == 0),
                        stop=(kc == 4 * qb + 3),
                    )
                    c = c2
                if kc >= 3 and (kc - 3) % 4 == 0:
                    qb_done = (kc - 3) // 4
                    if qlo <= qb_done * 512 < qhi:
                        normalize_qb(h, b, qb_done, ctx_ps[qb_done])

            # ---- per-(batch, half) gathers --------------------------------
            cc_in = {}
            cc_out = {}
            for b in range(B):
                for half in range(2):
                    cc_in[(b, half)] = dram.tile(
                        [128, 1024], BF16, name=f"cci_{b}_{half}"
                    )
                    cc_out[(b, half)] = dram.tile(
                        [NCORES * 128, 1024], BF16, addr_space="Shared",
                        name=f"cco_{b}_{half}",
                    )

            ctxF = {}

            def ship(b, half):
                t0 = b * S + half * 1024
                key = (b, half)
                for h in range(HPC):
                    nc.sync.dma_start(
                        cc_in[key][h * 64:h * 64 + 64, :],
                        ctxT_sb[:, h, t0:t0 + 1024],
                    )
                nc.gpsimd.collective_compute(
                    "AllGather",
                    mybir.AluOpType.bypass,
                    replica_groups=[list(range(NCORES))],
                    ins=[cc_in[key].opt()],
                    outs=[cc_out[key].opt()],
                )
                cf = cf_pool.tile([128, 8, 1024], BF16, tag="cf", name=f"cf_{b}_{half}")
                ctxF[key] = cf
                r = cc_out[key].rearrange("(o p) f -> p o f", p=128)
                for ki in range(8):
                    nc.sync.dma_start(cf[:, ki, :], r[:, ki, :])

            def outproj_half(b, half):
                ps = psA.tile([128, 1024], F32, tag="work", name=f"o_{b}_{half}")
                for ki in range(8):
                    for nb in range(2):
                        nc.tensor.matmul(
                            ps[:, nb * 512:nb * 512 + 512],
                            lhsT=w_sb["o"][:, ki, :],
                            rhs=ctxF[(b, half)][:, ki, nb * 512:nb * 512 + 512],
                            start=(ki == 0),
                            stop=(ki == 7 and not with_bias),
                        )
                if with_bias:
                    for nb in range(2):
                        nc.tensor.matmul(
                            ps[:, nb * 512:nb * 512 + 512],
                            lhsT=b_sb["bo"][0:1, :],
                            rhs=ones512[0:1, :],
                            start=False,
                            stop=True,
                        )
                ot = out_pool.tile([128, 1024], BF16, tag="out")
                nc.vector.tensor_copy(ot, ps)
                cs0 = b * S + half * 1024
                nc.sync.dma_start(outT_d[:, cs0:cs0 + 1024], ot)

            # ---- build the interleaved, software-pipelined stream ---------
            ctx_tiles = {}

            def get_ctx(p, qb):
                if (p, qb) not in ctx_tiles:
                    ctx_tiles[(p, qb)] = psB.tile(
                        [128, 512], F32, tag="ctx", name=f"cx_{p[0]}_{p[1]}_{qb}"
                    )
                return ctx_tiles[(p, qb)]

            def make_steps(p, pas):
                h, b = p
                qlo, qhi = (0, 1024) if pas == 0 else (1024, 2048)
                kcs = range(8) if pas == 0 else range(KC)
                return [(p, kc, qlo, qhi) for kc in kcs]

            pending = deque()

            def flush(n=None):
                k = len(pending) if n is None else n
                for _ in range(k):
                    pending.popleft()()

            def do_kc(arg):
                p, kc, qlo, qhi = arg
                h, b = p
                at = scores_part(h, b, kc, qlo, qhi)
                cps = {qb: get_ctx(p, qb) for qb in (qlo // 512, qlo // 512 + 1)}
                pending.append(
                    lambda h=h, b=b, kc=kc, qlo=qlo, qhi=qhi, cps=cps, at=at:
                    ctx_part(h, b, kc, qlo, qhi, cps, at)
                )
                # flush ctx in same-shape blocks of two (one per head),
                # lagging the scores so exp() has landed
                if len(pending) > 5:
                    flush(2)

            # batch-0 projections + v-transposes up front
            for which in ("v", "k", "q"):
                project(which, 0)
                project(which, 1)
                if which == "v":
                    vt_chunk(0)
                    vt_chunk(8)
            xT_frees[1]()  # xT batch 0

            # batch-0 attention (24 zipped kc-steps) with batch-1 prep
            # interleaved: each misc entry is a ~2-3.5us PE chunk slotted
            # between kc steps (the stream is ACT-bound, so these fill
            # Tensor-engine slack without starving exp).
            b0_misc = {
                2: lambda: project("v", 2),
                5: lambda: project("v", 3),
                8: lambda: vt_chunk(16),
                11: lambda: vt_chunk(24),
                14: lambda: project("k", 2),
                17: lambda: project("k", 3),
                20: lambda: project("q", 2),
                22: lambda: project("q", 3),
            }
            steps_b0 = list(zip(
                make_steps((0, 0), 0) + make_steps((0, 0), 1),
                make_steps((1, 0), 0) + make_steps((1, 0), 1),
            ))
            for j, (x, y) in enumerate(steps_b0, start=1):
                do_kc(x)
                do_kc(y)
                if j == 8:
                    flush()
                    ship(0, 0)
                if j in b0_misc:
                    b0_misc[j]()
            flush()
            ship(0, 1)
            xT_frees[0]()  # xT batch 1
            vT_free()

            # batch-1 attention with batch-0 out-proj interleaved (late
            # enough that the serialized CC stream has finished the
            # corresponding batch-0 gather).
            steps_b1 = list(zip(
                make_steps((0, 1), 0) + make_steps((0, 1), 1),
                make_steps((1, 1), 0) + make_steps((1, 1), 1),
            ))
            for j, (x, y) in enumerate(steps_b1, start=1):
                do_kc(x)
                do_kc(y)
                if j == 8:
                    flush()
                    ship(1, 0)
                elif j == 12:
                    outproj_half(0, 0)
                elif j == 17:
                    outproj_half(0, 1)
                elif j == 23:
                    # emitted before ship(1,1)'s DMA batch reaches the sync
                    # queue, so it isn't serialized behind the last gather
                    outproj_half(1, 0)
            flush()
            ship(1, 1)
            outproj_half(1, 1)

            ctxT_free()
            kT_free()
            qT_free()

    nc.compile()
    return nc


def _get_program(with_bias: bool = False):
    key = ("nc", with_bias)
    if key not in _CACHED:
        _CACHED[key] = _build(with_bias)
    return _CACHED[key]


# gathered feature order: [core r, local-head h, dh] -> global feature
# global head of (r, h) is 2r + h, so feature index = (2r + h) * DH + dh
_PERM = np.array(
    [(2 * r + h) * DH + dh for r in range(NCORES) for h in range(HPC) for dh in range(DH)]
)


def kernel(x, mask, wq, bq, wk, bk, wv, bv, wo, bo):
    x = np.asarray(x, dtype=np.float32)
    mask = np.asarray(mask)
    bf = ml_dtypes.bfloat16

    with_bias = any(np.any(np.asarray(bb)) for bb in (bq, bk, bv, bo))
    nc = _get_program(with_bias)

    # [feature, batch*seq] activations
    xT = np.ascontiguousarray(x.reshape(T, D).T).astype(bf)
    # -3.25: constant score shift so exp() fits fp8 e4m3 range (max score
    # in-distribution ~8.7 -> exp(5.45) ~ 232 < 448); softmax-invariant.
    maskb = np.ascontiguousarray(
        (np.where(np.asarray(mask).reshape(B * KC, 128), -10000.0, 0.0) - 3.25)
        .astype(np.float32)
        .T
    )
    in_maps = []
    for c in range(NCORES):
        fs = slice(c * F, (c + 1) * F)
        m = {
            "xT": xT,
            "wqT": np.ascontiguousarray(np.asarray(wq)[fs, :].T).astype(bf),
            "wkT": np.ascontiguousarray(np.asarray(wk)[fs, :].T).astype(bf),
            "wvT": np.ascontiguousarray(np.asarray(wv)[fs, :].T).astype(bf),
            "woT": np.ascontiguousarray(
                np.asarray(wo)[fs, :].T[_PERM]
            ).astype(bf),
            "maskb": maskb,
        }
        if with_bias:
            m["bq"] = np.asarray(bq)[fs].astype(bf).reshape(1, F)
            m["bk"] = np.asarray(bk)[fs].astype(bf).reshape(1, F)
            m["bv"] = np.asarray(bv)[fs].astype(bf).reshape(1, F)
            m["bo"] = np.asarray(bo)[fs].astype(bf).reshape(1, F)
        in_maps.append(m)

    res = bass_utils.run_bass_kernel_spmd(
        nc, in_maps, core_ids=list(range(NCORES)), trace=False
    )
    _CACHED["last_results"] = res

    out = np.empty((B, S, D), dtype=np.float32)
    for c in range(NCORES):
        o = np.asarray(res.results[c]["outT"], dtype=np.float32)  # [F, T]
        out[:, :, c * F:(c + 1) * F] = o.T.reshape(B, S, F)
    return out
